# revision 7
# baseline (speedup 1.0000x reference)
"""Raw-bass v9 "SYM": symmetry-halved pairwise work + PLAN-X pipeline.

Each core computes its 64 rows against rolled cols [0:288) (all rows) and
[288:320) (rows 32..63 only) — 0.59x the pairs of the full 512-col sweep.
Every pair {u, v} is covered exactly once across cores via the quadrant
rule validated in the host combiner: row-sums (Act accum) plus per-block
column-sums (DVE tensor_reduce over the materialized exp block) are
DMA'd out raw; the host assembles f and applies bias - 1.
"""

import numpy as np
from contextlib import ExitStack

B, NIN, NK, DK = 512, 1024, 128, 5
NCORES = 8
BL = B // NCORES
P = 128
IT = NIN // P
R = 8
NBLK = BL // R
G = 4  # number of (d<4) tiles
KG = NK // G  # 32 k per tile
RD = 6  # rows per block of tile-3 computed on DVE (rest on Act)
RA = 2  # rows per block of the d4-plane computed on DVE (rest on Act)
CE = 320  # extended col count (blocks 4..7)
CM = 288  # main col count (blocks 0..3)
C0 = 256
OUTW = 64 + 64 + 32 + 8 * 192 + 8 * 32 + 4 * 32  # 2080


def _register_absdiff():
    import concourse.dve_ops as dve_ops_mod

    if "ABS_DIFF_ANT" in dve_ops_mod._SUB_OPCODE_FOR_NAME:
        return next(o for o in dve_ops_mod.OPS if o.name == "ABS_DIFF_ANT")
    from concourse.dve_spec import Spec, Src0, Src1, maxx, lower
    from concourse.dve_uop import DveOpSpec
    from concourse.dve_ops import DveOp, _COMPILE_CACHE

    spec = Spec(
        body=maxx(Src0 - Src1, Src1 - Src0),
        reference=lambda in0, in1, s0, s1, imm2: np.abs(
            in0.astype(np.float32) - in1.astype(np.float32)
        ),
    )
    row = dve_ops_mod._CUSTOM_DVE_ROW_BASE + len(dve_ops_mod.OPS)
    op = DveOp("ABS_DIFF_ANT", spec, subdim=False, uops_sha={})
    dve_ops_mod.OPS.append(op)
    dve_ops_mod.CUSTOM_DVE_SPECS[op.name] = spec
    dve_ops_mod._SUB_OPCODE_FOR_NAME[op.name] = row
    for ver in ("v3", "v4"):
        _COMPILE_CACHE[(op.name, ver)] = DveOpSpec(
            name=op.name,
            opcode=row,
            uops=lower(spec, ver=ver),
            rd1_en=True,
        )
    return op


def _W(blk):
    return CM if blk < 4 else CE


def build_nc():
    import concourse.bacc as bacc
    import concourse.mybir as mybir

    ABS_DIFF = _register_absdiff()

    f32 = mybir.dt.float32
    bf16 = mybir.dt.bfloat16
    AF = mybir.ActivationFunctionType
    X = mybir.AxisListType.X
    OP = mybir.AluOpType

    nc = bacc.Bacc(None, target_bir_lowering=False)
    xT_d = nc.declare_dram_parameter("xTroll", [NIN, CE], bf16, isOutput=False)
    thp_d = nc.declare_dram_parameter("thp", [NIN, DK * NK + 1], bf16, isOutput=False)
    small_d = nc.declare_dram_parameter("small", [P, 6], f32, isOutput=False)
    sel_d = nc.declare_dram_parameter("sel", [P, KG], bf16, isOutput=False)
    eye_d = nc.declare_dram_parameter("eye", [P, P], bf16, isOutput=False)
    out_d = nc.declare_dram_parameter("fT", [P, OUTW], f32, isOutput=True)

    with ExitStack() as ctx:
        en = ctx.enter_context
        th_all = en(nc.sbuf_tensor([P, IT, DK * NK + 1], bf16))
        xT_all = en(nc.sbuf_tensor([P, IT, CE], bf16))
        th2 = en(nc.sbuf_tensor([P, IT, DK * NK + 1], bf16))
        small = en(nc.sbuf_tensor([P, 6], f32))
        sel = en(nc.sbuf_tensor([P, KG], bf16))
        eye = en(nc.sbuf_tensor([P, P], bf16))
        l2c = en(nc.sbuf_tensor([P, DK], f32))
        invc = en(nc.sbuf_tensor([P, DK], f32))
        elws = en(nc.sbuf_tensor([P, DK], f32))
        scale = en(nc.sbuf_tensor([P, DK], f32))
        avT4 = en(nc.sbuf_tensor([P, G, CE], bf16))
        avD4 = en(nc.sbuf_tensor([P, CE], bf16))
        negD4 = en(nc.sbuf_tensor([P, BL], f32))
        negT3 = en(nc.sbuf_tensor([P, BL], f32))
        ad = [en(nc.sbuf_tensor(f"ad{i}", [P, G, R, CE], bf16)) for i in range(2)]
        ad4 = [en(nc.sbuf_tensor(f"ad4{i}", [P, R, CE], bf16)) for i in range(2)]
        Eb = [en(nc.sbuf_tensor(f"Eb{i}", [P, R, CE], bf16)) for i in range(3)]
        cst = en(nc.sbuf_tensor([P, 4, 192], bf16))
        OUT = en(nc.sbuf_tensor([P, OUTW], f32))
        ps_av = [en(nc.psum_tensor(f"psav{g}", [P, CE], f32)) for g in range(G)]
        ps_av4 = en(nc.psum_tensor("psav4", [P, CE], f32))
        ps_sq = en(nc.psum_tensor("ps_sq", [P, DK], f32))
        psL = [en(nc.psum_tensor(f"psL{j}", [P, CE], f32)) for j in range(2)]

        with (
            nc.semaphore("dTH") as dTH,
            nc.semaphore("dSM") as dSM,
            nc.semaphore("dSE") as dSE,
            nc.semaphore("dOU") as dOU,
            nc.semaphore("dX") as dX,
            nc.semaphore("sQ") as sQ,
            nc.semaphore("sC") as sC,
            nc.semaphore("sA") as sA,
            nc.semaphore("sV") as sV,
            nc.semaphore("sD") as sD,
            nc.semaphore("sD4") as sD4,
            nc.semaphore("sL") as sL,
            nc.semaphore("sE") as sE,
            nc.semaphore("sTR") as sTR,
            nc.Block() as block,
        ):

            @block.sync
            def _(sync):
                # two halves -> two DMA engines run concurrently
                thr = thp_d.rearrange("(i p) c -> p i c", p=P)
                sync.dma_start(th_all[:, 0 : IT // 2, :], thr[:, 0 : IT // 2, :]).then_inc(dTH, 16)
                sync.dma_start(th_all[:, IT // 2 : IT, :], thr[:, IT // 2 : IT, :]).then_inc(dTH, 16)
                sync.dma_start(small[:], small_d[:, :]).then_inc(dSM, 16)
                sync.dma_start(sel[:], sel_d[:, :]).then_inc(dSE, 16)
                sync.dma_start(eye[:], eye_d[:, :]).then_inc(dSE, 16)
                sync.wait_ge(sE, BL)  # all row-sum accums done
                sync.wait_ge(sTR, NBLK)  # all col-sum reduces done
                sync.dma_start(out_d[:, :], OUT[:]).then_inc(dOU, 16)
                sync.wait_ge(dOU, 16)
                sync.wait_ge(dX, 16)

            @block.tensor
            def _(tensor):
                def actv_chain(cols, out_ps):
                    for i in range(IT):
                        mm = nc.tensor.matmul(
                            out_ps[:],
                            th_all[:, i, cols],
                            xT_all[:, i, :],
                            start=(i == 0),
                            stop=(i == IT - 1),
                        )
                    return mm

                def ssq_chain(cols, col):
                    for i in range(IT):
                        mm = nc.tensor.matmul(
                            ps_sq[:, col : col + 1],
                            th2[:, i, cols],
                            th_all[:, i, DK * NK : DK * NK + 1],
                            start=(i == 0),
                            stop=(i == IT - 1),
                        )
                    return mm

                # chain order: g0, g1, SSQ, d4, g2, g3
                tensor.wait_ge(dTH, 32)
                tensor.wait_ge(dX, 16)
                actv_chain(slice(0, P), ps_av[0]).then_inc(sC, 1)
                actv_chain(slice(P, 2 * P), ps_av[1]).then_inc(sC, 1)
                tensor.wait_ge(sQ, 1)  # th2 ready
                for g in range(G):
                    ssq_chain(slice(P * g, P * (g + 1)), g)
                ssq_chain(slice(G * P, G * P + NK), G).then_inc(sC, 1)
                actv_chain(slice(G * P, G * P + NK), ps_av4).then_inc(sC, 1)
                actv_chain(slice(2 * P, 3 * P), ps_av[2]).then_inc(sC, 1)
                actv_chain(slice(3 * P, 4 * P), ps_av[3]).then_inc(sC, 1)
                tensor.wait_ge(dSE, 32)  # sel + eye loaded
                for blk in range(NBLK):
                    W = _W(blk)
                    tensor.wait_ge(sD, blk + 1)  # absdiff tiles ready (DVE)
                    tensor.wait_ge(sD4, blk + 1)  # plane-4 + tile3 tail (Act)
                    buf = blk % 2
                    for r in range(R):
                        gr = blk * R + r
                        j = gr % 2
                        if gr >= 2:
                            tensor.wait_ge(sE, gr - 1)  # Act freed bank j
                        for g in range(G):
                            nc.tensor.matmul(
                                psL[j][KG * g : KG * (g + 1), 0:W],
                                sel[:, :],
                                ad[buf][:, g, r, 0:W],
                                start=True,
                                stop=False,
                                tile_position=(0, KG * g),
                            )
                        for g in range(G):
                            mm = nc.tensor.matmul(
                                psL[j][KG * g : KG * (g + 1), 0:W],
                                eye[:, KG * g : KG * (g + 1)],
                                ad4[buf][:, r, 0:W],
                                start=False,
                                stop=True,
                                tile_position=(0, KG * g),
                            )
                        mm.then_inc(sL, 1)

            @block.scalar
            def _(scalar):
                nc.scalar.dma_start(
                    xT_all[:], xT_d.rearrange("(i p) c -> p i c", p=P)
                ).then_inc(dX, 16)
                scalar.wait_ge(dTH, 32)
                nc.scalar.activation(th2[:], th_all[:], AF.Square).then_inc(
                    sQ, 1
                )
                scalar.wait_ge(sC, 3)  # ssq in psum
                nc.scalar.activation(l2c[:], ps_sq[:], AF.Sqrt).then_inc(sA, 1)
                scalar.wait_ge(dSM, 16)
                nc.scalar.activation(
                    elws[:], small[:, 0:DK], AF.Exp
                ).then_inc(sA, 1)
                # scaled PSUM->SBUF copies (chain order g0, g1, SSQ, d4, g2, g3)
                scalar.wait_ge(sV, 2)  # scale ready
                scalar.wait_ge(sC, 1)
                nc.scalar.activation(
                    avT4[:, 0, :], ps_av[0][:], AF.Copy, scale=scale[:, 0:1]
                )
                scalar.wait_ge(sC, 2)
                nc.scalar.activation(
                    avT4[:, 1, :], ps_av[1][:], AF.Copy, scale=scale[:, 1:2]
                )
                scalar.wait_ge(sC, 4)
                nc.scalar.activation(
                    avD4[:], ps_av4[:], AF.Copy, scale=scale[:, G : G + 1]
                )
                nc.scalar.activation(
                    negD4[:], avD4[:, 0:BL], AF.Copy, scale=-1.0
                )
                scalar.wait_ge(sC, 5)
                nc.scalar.activation(
                    avT4[:, 2, :], ps_av[2][:], AF.Copy, scale=scale[:, 2:3]
                )
                scalar.wait_ge(sC, 6)
                nc.scalar.activation(
                    avT4[:, 3, :], ps_av[3][:], AF.Copy, scale=scale[:, 3:4]
                )
                act = nc.scalar.activation(
                    negT3[:], avT4[:, 3, 0:BL], AF.Copy, scale=-1.0
                )
                act.then_inc(sA, 1)  # sA=3: avT4/avD4/negs ready

                def emit_abs(blk):
                    W = _W(blk)
                    if blk >= 2:
                        scalar.wait_ge(sL, (blk - 1) * R)  # PE freed bufs
                    for r in range(RA, R):
                        gr = blk * R + r
                        nc.scalar.activation(
                            ad4[blk % 2][:, r, 0:W],
                            avD4[:, 0:W],
                            AF.Abs,
                            bias=negD4[:, gr : gr + 1],
                        )
                    for r in range(RD, R):
                        gr = blk * R + r
                        act = nc.scalar.activation(
                            ad[blk % 2][:, 3, r, 0:W],
                            avT4[:, 3, 0:W],
                            AF.Abs,
                            bias=negT3[:, gr : gr + 1],
                        )
                    act.then_inc(sD4, 1)

                def emit_exps(blk):
                    W = _W(blk)
                    if blk >= 3:
                        scalar.wait_ge(sTR, blk - 2)  # Eb[blk%3] reduced
                    for r in range(R):
                        gr = blk * R + r
                        scalar.wait_ge(sL, gr + 1)
                        nc.scalar.activation(
                            Eb[blk % 3][:, r, 0:W],
                            psL[gr % 2][:, 0:W],
                            AF.Exp,
                            scale=-1.0,
                            accum_out=OUT[:, gr : gr + 1],
                        )
                        act = nc.scalar.activation(
                            EscF[:],
                            Eb[blk % 3][:, r, C0:CM],
                            AF.Identity,
                            accum_out=OUT[:, 64 + gr : 64 + gr + 1],
                        )
                        act.then_inc(sE, 1)

                for blk in range(NBLK):
                    emit_abs(blk)
                    if blk >= 1:
                        emit_exps(blk - 1)
                emit_exps(NBLK - 1)

            @block.vector
            def _(vector):
                vector.wait_ge(sA, 1)
                nc.vector.reciprocal(invc[:], l2c[:])
                vector.wait_ge(sA, 2)
                nc.vector.tensor_mul(scale[:], elws[:], invc[:]).then_inc(
                    sV, 2
                )

                def emit_colsums(b):
                    # reduce Eb[b%3] for block b: row-sums of col segments
                    # (fA/fB) and per-block column-sums (csM/csA/csB).
                    e = Eb[b % 3][:]
                    vector.wait_ge(sE, (b + 1) * R)  # Act exps of b done
                    if b >= 4:
                        nc.vector.tensor_reduce(
                            OUT[:, 128 + 8 * (b - 4) : 128 + 8 * (b - 4) + 8],
                            e[:, :, CM:CE],
                            axis=X,
                            op=OP.add,
                        )
                    # csM via contiguous bf16 pairwise adds (2x mode) instead
                    # of a strided-inner tensor_reduce (1.8 ns/elem).
                    for t in range(4):
                        nc.vector.tensor_tensor(
                            out=cst[:, t, :],
                            in0=e[:, 2 * t, 64:C0],
                            in1=e[:, 2 * t + 1, 64:C0],
                            op=OP.add,
                        )
                    nc.vector.tensor_tensor(
                        out=cst[:, 0, :], in0=cst[:, 0, :], in1=cst[:, 1, :],
                        op=OP.add,
                    )
                    nc.vector.tensor_tensor(
                        out=cst[:, 2, :], in0=cst[:, 2, :], in1=cst[:, 3, :],
                        op=OP.add,
                    )
                    nc.vector.tensor_tensor(
                        out=OUT[:, 160 + 192 * b : 160 + 192 * (b + 1)],
                        in0=cst[:, 0, :],
                        in1=cst[:, 2, :],
                        op=OP.add,
                    )
                    inst = nc.vector.tensor_reduce(
                        OUT[:, 1696 + 32 * b : 1696 + 32 * (b + 1)],
                        e[:, :, C0:CM].rearrange("p r c -> p c r"),
                        axis=X,
                        op=OP.add,
                    )
                    if b >= 4:
                        inst = nc.vector.tensor_reduce(
                            OUT[:, 1952 + 32 * (b - 4) : 1952 + 32 * (b - 3)],
                            e[:, :, CM:CE].rearrange("p r c -> p c r"),
                            axis=X,
                            op=OP.add,
                        )
                    inst.then_inc(sTR, 1)

                vector.wait_ge(sA, 3)  # avT4 ready
                for blk in range(NBLK):
                    W = _W(blk)
                    r0 = blk * R
                    buf = blk % 2
                    if blk >= 2:
                        vector.wait_ge(sL, (blk - 1) * R)
                    for g in range(3):
                        nc.vector._custom_dve(
                            ABS_DIFF,
                            out=ad[buf][:, g, :, 0:W],
                            in0=avT4[:][:, g, None, 0:W].broadcast_to(
                                [P, R, W]
                            ),
                            in1=avT4[:][:, g, r0 : r0 + R, None].broadcast_to(
                                [P, R, W]
                            ),
                        )
                    nc.vector._custom_dve(
                        ABS_DIFF,
                        out=ad[buf][:, 3, 0:RD, 0:W],
                        in0=avT4[:][:, 3, None, 0:W].broadcast_to([P, RD, W]),
                        in1=avT4[:][:, 3, r0 : r0 + RD, None].broadcast_to(
                            [P, RD, W]
                        ),
                    )
                    inst = nc.vector._custom_dve(
                        ABS_DIFF,
                        out=ad4[buf][:, 0:RA, 0:W],
                        in0=avD4[:][:, None, 0:W].broadcast_to([P, RA, W]),
                        in1=avD4[:][:, r0 : r0 + RA, None].broadcast_to(
                            [P, RA, W]
                        ),
                    )
                    inst.then_inc(sD, 1)
                    if blk >= 2:
                        emit_colsums(blk - 2)
                emit_colsums(NBLK - 2)
                emit_colsums(NBLK - 1)

    nc.compile()
    return nc


def _perm():
    ks = np.array(
        [KG * g + (p % KG) for g in range(G) for p in range(P)]
        + list(range(NK))
    )
    ds = np.array([p // KG for g in range(G) for p in range(P)] + [4] * NK)
    return ks, ds


def make_in_maps(x, theta, log_weight_scale, bias):
    import ml_dtypes

    bf = ml_dtypes.bfloat16
    xT = np.ascontiguousarray(x.T).astype(bf)
    ks, ds = _perm()
    thp = np.ascontiguousarray(theta[:, ks, ds]).astype(bf)  # [NIN, 640]
    thp = np.concatenate([thp, np.ones((NIN, 1), dtype=bf)], axis=1)
    small = np.zeros((P, 6), dtype=np.float32)
    pp = np.arange(P)
    for g in range(G):
        small[:, g] = log_weight_scale[KG * g + (pp % KG), pp // KG]
    small[:, G] = log_weight_scale[:, 4]
    small[:, 5] = bias
    sel = (pp[:, None] % KG == np.arange(KG)[None, :]).astype(bf)
    eye = np.eye(P, dtype=bf)
    return [
        {
            "xTroll": np.ascontiguousarray(
                np.roll(xT, -BL * c, axis=1)[:, 0:CE]
            ),
            "thp": thp,
            "small": small,
            "sel": sel,
            "eye": eye,
        }
        for c in range(NCORES)
    ]


_CACHE = {}


def get_nc():
    if "nc" not in _CACHE:
        _CACHE["nc"] = build_nc()
    return _CACHE["nc"]


def kernel(x, theta, log_weight_scale, bias):
    from concourse.bass_utils import run_bass_kernel_spmd

    x = np.asarray(x, dtype=np.float32)
    theta = np.asarray(theta, dtype=np.float32)
    log_weight_scale = np.asarray(log_weight_scale, dtype=np.float32)
    bias = np.asarray(bias, dtype=np.float32)

    nc = get_nc()
    in_maps = make_in_maps(x, theta, log_weight_scale, bias)
    res = run_bass_kernel_spmd(nc, in_maps, list(range(NCORES))).results

    f = np.zeros((B, NK), dtype=np.float64)
    for c in range(NCORES):
        O = res[c]["fT"].astype(np.float64)  # [128, OUTW]
        fMA = O[:, 0:64]
        fA = O[:, 64:128]
        fB = O[:, 128:160]
        csM = O[:, 160:1696].reshape(NK, 8, 192)
        csA = O[:, 1696:1952].reshape(NK, 8, 32)
        csB = O[:, 1952:2080].reshape(NK, 4, 32)
        rows = (BL * c + np.arange(BL)) % B
        contrib = fMA.copy()
        if c >= 4:
            contrib[:, 0:32] -= fA[:, 0:32]
            contrib[:, 32:64] -= fB
        f[rows] += contrib.T
        f[(BL * c + 64 + np.arange(192)) % B] += csM.sum(axis=1).T
        vA = (BL * c + C0 + np.arange(32)) % B
        f[vA] += csA[:, 4:8].sum(axis=1).T
        if c < 4:
            f[vA] += csA[:, 0:4].sum(axis=1).T
            f[(BL * c + CM + np.arange(32)) % B] += csB.sum(axis=1).T
    f = f + bias[None, :] - 1.0
    return np.concatenate([x, f.astype(np.float32)], axis=1)


# revision 8
# speedup vs baseline: 1.0085x; 1.0085x over previous
"""Raw-bass v9 "SYM": symmetry-halved pairwise work + PLAN-X pipeline.

Each core computes its 64 rows against rolled cols [0:288) (all rows) and
[288:320) (rows 32..63 only) — 0.59x the pairs of the full 512-col sweep.
Every pair {u, v} is covered exactly once across cores via the quadrant
rule validated in the host combiner: row-sums (Act accum) plus per-block
column-sums (DVE tensor_reduce over the materialized exp block) are
DMA'd out raw; the host assembles f and applies bias - 1.
"""

import numpy as np
from contextlib import ExitStack

B, NIN, NK, DK = 512, 1024, 128, 5
NCORES = 8
BL = B // NCORES
P = 128
IT = NIN // P
R = 8
NBLK = BL // R
G = 4  # number of (d<4) tiles
KG = NK // G  # 32 k per tile
RD = 6  # rows per block of tile-3 computed on DVE (rest on Act)
RA = 2  # rows per block of the d4-plane computed on DVE (rest on Act)
CE = 320  # extended col count (blocks 4..7)
CM = 288  # main col count (blocks 0..3)
C0 = 256
OUTW = 64 + 64 + 32 + 8 * 192 + 8 * 32 + 4 * 32  # 2080


def _register_absdiff():
    import concourse.dve_ops as dve_ops_mod

    if "ABS_DIFF_ANT" in dve_ops_mod._SUB_OPCODE_FOR_NAME:
        return next(o for o in dve_ops_mod.OPS if o.name == "ABS_DIFF_ANT")
    from concourse.dve_spec import Spec, Src0, Src1, maxx, lower
    from concourse.dve_uop import DveOpSpec
    from concourse.dve_ops import DveOp, _COMPILE_CACHE

    spec = Spec(
        body=maxx(Src0 - Src1, Src1 - Src0),
        reference=lambda in0, in1, s0, s1, imm2: np.abs(
            in0.astype(np.float32) - in1.astype(np.float32)
        ),
    )
    row = dve_ops_mod._CUSTOM_DVE_ROW_BASE + len(dve_ops_mod.OPS)
    op = DveOp("ABS_DIFF_ANT", spec, subdim=False, uops_sha={})
    dve_ops_mod.OPS.append(op)
    dve_ops_mod.CUSTOM_DVE_SPECS[op.name] = spec
    dve_ops_mod._SUB_OPCODE_FOR_NAME[op.name] = row
    for ver in ("v3", "v4"):
        _COMPILE_CACHE[(op.name, ver)] = DveOpSpec(
            name=op.name,
            opcode=row,
            uops=lower(spec, ver=ver),
            rd1_en=True,
        )
    return op


def _W(blk):
    return CM if blk < 4 else CE


def build_nc():
    import concourse.bacc as bacc
    import concourse.mybir as mybir

    ABS_DIFF = _register_absdiff()

    f32 = mybir.dt.float32
    bf16 = mybir.dt.bfloat16
    AF = mybir.ActivationFunctionType
    X = mybir.AxisListType.X
    OP = mybir.AluOpType

    nc = bacc.Bacc(None, target_bir_lowering=False)
    xT_d = nc.declare_dram_parameter("xTroll", [NIN, CE], bf16, isOutput=False)
    thp_d = nc.declare_dram_parameter("thp", [NIN, DK * NK + 1], bf16, isOutput=False)
    small_d = nc.declare_dram_parameter("small", [P, 6], f32, isOutput=False)
    sel_d = nc.declare_dram_parameter("sel", [P, KG], bf16, isOutput=False)
    eye_d = nc.declare_dram_parameter("eye", [P, P], bf16, isOutput=False)
    out_d = nc.declare_dram_parameter("fT", [P, OUTW], f32, isOutput=True)

    with ExitStack() as ctx:
        en = ctx.enter_context
        th_all = en(nc.sbuf_tensor([P, IT, DK * NK + 1], bf16))
        xT_all = en(nc.sbuf_tensor([P, IT, CE], bf16))
        th2 = en(nc.sbuf_tensor([P, IT, DK * NK + 1], bf16))
        small = en(nc.sbuf_tensor([P, 6], f32))
        sel = en(nc.sbuf_tensor([P, KG], bf16))
        eye = en(nc.sbuf_tensor([P, P], bf16))
        l2c = en(nc.sbuf_tensor([P, DK], f32))
        invc = en(nc.sbuf_tensor([P, DK], f32))
        elws = en(nc.sbuf_tensor([P, DK], f32))
        scale = en(nc.sbuf_tensor([P, DK], f32))
        avT4 = en(nc.sbuf_tensor([P, G, CE], bf16))
        avD4 = en(nc.sbuf_tensor([P, CE], bf16))
        negD4 = en(nc.sbuf_tensor([P, BL], f32))
        negT3 = en(nc.sbuf_tensor([P, BL], f32))
        ad = [en(nc.sbuf_tensor(f"ad{i}", [P, G, R, CE], bf16)) for i in range(2)]
        ad4 = [en(nc.sbuf_tensor(f"ad4{i}", [P, R, CE], bf16)) for i in range(2)]
        Eb = [en(nc.sbuf_tensor(f"Eb{i}", [P, R, CE], bf16)) for i in range(3)]
        cst = en(nc.sbuf_tensor([P, 4, 192], bf16))
        OUT = en(nc.sbuf_tensor([P, OUTW], f32))
        ps_av = [en(nc.psum_tensor(f"psav{g}", [P, CE], f32)) for g in range(G)]
        ps_av4 = en(nc.psum_tensor("psav4", [P, CE], f32))
        ps_sq = en(nc.psum_tensor("ps_sq", [P, DK], f32))
        psL = [en(nc.psum_tensor(f"psL{j}", [P, CE], f32)) for j in range(2)]

        with (
            nc.semaphore("dTH") as dTH,
            nc.semaphore("dSM") as dSM,
            nc.semaphore("dSE") as dSE,
            nc.semaphore("dOU") as dOU,
            nc.semaphore("dX") as dX,
            nc.semaphore("sQ") as sQ,
            nc.semaphore("sC") as sC,
            nc.semaphore("sA") as sA,
            nc.semaphore("sV") as sV,
            nc.semaphore("sD") as sD,
            nc.semaphore("sD4") as sD4,
            nc.semaphore("sL") as sL,
            nc.semaphore("sE") as sE,
            nc.semaphore("sTR") as sTR,
            nc.Block() as block,
        ):

            @block.sync
            def _(sync):
                # two halves -> two DMA engines run concurrently
                thr = thp_d.rearrange("(i p) c -> p i c", p=P)
                sync.dma_start(th_all[:, 0 : IT // 2, :], thr[:, 0 : IT // 2, :]).then_inc(dTH, 16)
                sync.dma_start(th_all[:, IT // 2 : IT, :], thr[:, IT // 2 : IT, :]).then_inc(dTH, 16)
                sync.dma_start(small[:], small_d[:, :]).then_inc(dSM, 16)
                sync.dma_start(sel[:], sel_d[:, :]).then_inc(dSE, 16)
                sync.dma_start(eye[:], eye_d[:, :]).then_inc(dSE, 16)
                sync.wait_ge(sE, BL)  # all row-sum accums done
                sync.wait_ge(sTR, NBLK)  # all col-sum reduces done
                sync.dma_start(out_d[:, :], OUT[:]).then_inc(dOU, 16)
                sync.wait_ge(dOU, 16)
                sync.wait_ge(dX, 16)

            @block.tensor
            def _(tensor):
                def actv_chain(cols, out_ps):
                    for i in range(IT):
                        mm = nc.tensor.matmul(
                            out_ps[:],
                            th_all[:, i, cols],
                            xT_all[:, i, :],
                            start=(i == 0),
                            stop=(i == IT - 1),
                        )
                    return mm

                def ssq_chain(cols, col):
                    for i in range(IT):
                        mm = nc.tensor.matmul(
                            ps_sq[:, col : col + 1],
                            th2[:, i, cols],
                            th_all[:, i, DK * NK : DK * NK + 1],
                            start=(i == 0),
                            stop=(i == IT - 1),
                        )
                    return mm

                # chain order: g0, g1, SSQ, d4, g2, g3
                tensor.wait_ge(dTH, 32)
                tensor.wait_ge(dX, 16)
                actv_chain(slice(0, P), ps_av[0]).then_inc(sC, 1)
                actv_chain(slice(P, 2 * P), ps_av[1]).then_inc(sC, 1)
                tensor.wait_ge(sQ, 1)  # th2 ready
                for g in range(G):
                    ssq_chain(slice(P * g, P * (g + 1)), g)
                ssq_chain(slice(G * P, G * P + NK), G).then_inc(sC, 1)
                actv_chain(slice(G * P, G * P + NK), ps_av4).then_inc(sC, 1)
                actv_chain(slice(2 * P, 3 * P), ps_av[2]).then_inc(sC, 1)
                actv_chain(slice(3 * P, 4 * P), ps_av[3]).then_inc(sC, 1)
                tensor.wait_ge(dSE, 32)  # sel + eye loaded
                for blk in range(NBLK):
                    W = _W(blk)
                    tensor.wait_ge(sD, blk + 1)  # absdiff tiles ready (DVE)
                    tensor.wait_ge(sD4, blk + 1)  # plane-4 + tile3 tail (Act)
                    buf = blk % 2
                    for r in range(R):
                        gr = blk * R + r
                        j = gr % 2
                        if gr >= 2:
                            tensor.wait_ge(sE, gr - 1)  # Act freed bank j
                        for g in range(G):
                            nc.tensor.matmul(
                                psL[j][KG * g : KG * (g + 1), 0:W],
                                sel[:, :],
                                ad[buf][:, g, r, 0:W],
                                start=True,
                                stop=False,
                                tile_position=(0, KG * g),
                            )
                        for g in range(G):
                            mm = nc.tensor.matmul(
                                psL[j][KG * g : KG * (g + 1), 0:W],
                                eye[:, KG * g : KG * (g + 1)],
                                ad4[buf][:, r, 0:W],
                                start=False,
                                stop=True,
                                tile_position=(0, KG * g),
                            )
                        mm.then_inc(sL, 1)

            @block.scalar
            def _(scalar):
                nc.scalar.dma_start(
                    xT_all[:], xT_d.rearrange("(i p) c -> p i c", p=P)
                ).then_inc(dX, 16)
                scalar.wait_ge(dTH, 32)
                nc.scalar.activation(th2[:], th_all[:], AF.Square).then_inc(
                    sQ, 1
                )
                scalar.wait_ge(sC, 3)  # ssq in psum
                nc.scalar.activation(l2c[:], ps_sq[:], AF.Sqrt).then_inc(sA, 1)
                scalar.wait_ge(dSM, 16)
                nc.scalar.activation(
                    elws[:], small[:, 0:DK], AF.Exp
                ).then_inc(sA, 1)
                # scaled PSUM->SBUF copies (chain order g0, g1, SSQ, d4, g2, g3)
                scalar.wait_ge(sV, 2)  # scale ready
                scalar.wait_ge(sC, 1)
                nc.scalar.activation(
                    avT4[:, 0, :], ps_av[0][:], AF.Copy, scale=scale[:, 0:1]
                )
                scalar.wait_ge(sC, 2)
                nc.scalar.activation(
                    avT4[:, 1, :], ps_av[1][:], AF.Copy, scale=scale[:, 1:2]
                )
                scalar.wait_ge(sC, 4)
                nc.scalar.activation(
                    avD4[:], ps_av4[:], AF.Copy, scale=scale[:, G : G + 1]
                )
                nc.scalar.activation(
                    negD4[:], avD4[:, 0:BL], AF.Copy, scale=-1.0
                )
                scalar.wait_ge(sC, 5)
                nc.scalar.activation(
                    avT4[:, 2, :], ps_av[2][:], AF.Copy, scale=scale[:, 2:3]
                )
                scalar.wait_ge(sC, 6)
                nc.scalar.activation(
                    avT4[:, 3, :], ps_av[3][:], AF.Copy, scale=scale[:, 3:4]
                )
                act = nc.scalar.activation(
                    negT3[:], avT4[:, 3, 0:BL], AF.Copy, scale=-1.0
                )
                act.then_inc(sA, 1)  # sA=3: avT4/avD4/negs ready

                def emit_abs(blk):
                    W = _W(blk)
                    if blk >= 2:
                        scalar.wait_ge(sL, (blk - 1) * R)  # PE freed bufs
                    for r in range(RA, R):
                        gr = blk * R + r
                        nc.scalar.activation(
                            ad4[blk % 2][:, r, 0:W],
                            avD4[:, 0:W],
                            AF.Abs,
                            bias=negD4[:, gr : gr + 1],
                        )
                    for r in range(RD, R):
                        gr = blk * R + r
                        act = nc.scalar.activation(
                            ad[blk % 2][:, 3, r, 0:W],
                            avT4[:, 3, 0:W],
                            AF.Abs,
                            bias=negT3[:, gr : gr + 1],
                        )
                    act.then_inc(sD4, 1)

                def emit_exps(blk):
                    W = _W(blk)
                    if blk >= 3:
                        scalar.wait_ge(sTR, blk - 2)  # Eb[blk%3] reduced
                    for r in range(R):
                        gr = blk * R + r
                        scalar.wait_ge(sL, gr + 1)
                        act = nc.scalar.activation(
                            Eb[blk % 3][:, r, 0:W],
                            psL[gr % 2][:, 0:W],
                            AF.Exp,
                            scale=-1.0,
                            accum_out=OUT[:, gr : gr + 1],
                        )
                        act.then_inc(sE, 1)

                for blk in range(NBLK):
                    emit_abs(blk)
                    if blk >= 1:
                        emit_exps(blk - 1)
                emit_exps(NBLK - 1)

            @block.vector
            def _(vector):
                vector.wait_ge(sA, 1)
                nc.vector.reciprocal(invc[:], l2c[:])
                vector.wait_ge(sA, 2)
                nc.vector.tensor_mul(scale[:], elws[:], invc[:]).then_inc(
                    sV, 2
                )

                def emit_colsums(b):
                    # reduce Eb[b%3] for block b: row-sums of col segments
                    # (fA/fB) and per-block column-sums (csM/csA/csB).
                    e = Eb[b % 3][:]
                    vector.wait_ge(sE, (b + 1) * R)  # Act exps of b done
                    nc.vector.tensor_reduce(
                        OUT[:, 64 + 8 * b : 64 + 8 * b + 8],
                        e[:, :, C0:CM],
                        axis=X,
                        op=OP.add,
                    )
                    if b >= 4:
                        nc.vector.tensor_reduce(
                            OUT[:, 128 + 8 * (b - 4) : 128 + 8 * (b - 4) + 8],
                            e[:, :, CM:CE],
                            axis=X,
                            op=OP.add,
                        )
                    # csM via contiguous bf16 pairwise adds (2x mode) instead
                    # of a strided-inner tensor_reduce (1.8 ns/elem).
                    for t in range(4):
                        nc.vector.tensor_tensor(
                            out=cst[:, t, :],
                            in0=e[:, 2 * t, 64:C0],
                            in1=e[:, 2 * t + 1, 64:C0],
                            op=OP.add,
                        )
                    nc.vector.tensor_tensor(
                        out=cst[:, 0, :], in0=cst[:, 0, :], in1=cst[:, 1, :],
                        op=OP.add,
                    )
                    nc.vector.tensor_tensor(
                        out=cst[:, 2, :], in0=cst[:, 2, :], in1=cst[:, 3, :],
                        op=OP.add,
                    )
                    nc.vector.tensor_tensor(
                        out=OUT[:, 160 + 192 * b : 160 + 192 * (b + 1)],
                        in0=cst[:, 0, :],
                        in1=cst[:, 2, :],
                        op=OP.add,
                    )
                    inst = nc.vector.tensor_reduce(
                        OUT[:, 1696 + 32 * b : 1696 + 32 * (b + 1)],
                        e[:, :, C0:CM].rearrange("p r c -> p c r"),
                        axis=X,
                        op=OP.add,
                    )
                    if b >= 4:
                        inst = nc.vector.tensor_reduce(
                            OUT[:, 1952 + 32 * (b - 4) : 1952 + 32 * (b - 3)],
                            e[:, :, CM:CE].rearrange("p r c -> p c r"),
                            axis=X,
                            op=OP.add,
                        )
                    inst.then_inc(sTR, 1)

                vector.wait_ge(sA, 3)  # avT4 ready
                for blk in range(NBLK):
                    W = _W(blk)
                    r0 = blk * R
                    buf = blk % 2
                    if blk >= 2:
                        vector.wait_ge(sL, (blk - 1) * R)
                    for g in range(3):
                        nc.vector._custom_dve(
                            ABS_DIFF,
                            out=ad[buf][:, g, :, 0:W],
                            in0=avT4[:][:, g, None, 0:W].broadcast_to(
                                [P, R, W]
                            ),
                            in1=avT4[:][:, g, r0 : r0 + R, None].broadcast_to(
                                [P, R, W]
                            ),
                        )
                    nc.vector._custom_dve(
                        ABS_DIFF,
                        out=ad[buf][:, 3, 0:RD, 0:W],
                        in0=avT4[:][:, 3, None, 0:W].broadcast_to([P, RD, W]),
                        in1=avT4[:][:, 3, r0 : r0 + RD, None].broadcast_to(
                            [P, RD, W]
                        ),
                    )
                    inst = nc.vector._custom_dve(
                        ABS_DIFF,
                        out=ad4[buf][:, 0:RA, 0:W],
                        in0=avD4[:][:, None, 0:W].broadcast_to([P, RA, W]),
                        in1=avD4[:][:, r0 : r0 + RA, None].broadcast_to(
                            [P, RA, W]
                        ),
                    )
                    inst.then_inc(sD, 1)
                    if blk >= 2:
                        emit_colsums(blk - 2)
                emit_colsums(NBLK - 2)
                emit_colsums(NBLK - 1)

    nc.compile()
    return nc


def _perm():
    ks = np.array(
        [KG * g + (p % KG) for g in range(G) for p in range(P)]
        + list(range(NK))
    )
    ds = np.array([p // KG for g in range(G) for p in range(P)] + [4] * NK)
    return ks, ds


def make_in_maps(x, theta, log_weight_scale, bias):
    import ml_dtypes

    bf = ml_dtypes.bfloat16
    xT = np.ascontiguousarray(x.T).astype(bf)
    ks, ds = _perm()
    thp = np.ascontiguousarray(theta[:, ks, ds]).astype(bf)  # [NIN, 640]
    thp = np.concatenate([thp, np.ones((NIN, 1), dtype=bf)], axis=1)
    small = np.zeros((P, 6), dtype=np.float32)
    pp = np.arange(P)
    for g in range(G):
        small[:, g] = log_weight_scale[KG * g + (pp % KG), pp // KG]
    small[:, G] = log_weight_scale[:, 4]
    small[:, 5] = bias
    sel = (pp[:, None] % KG == np.arange(KG)[None, :]).astype(bf)
    eye = np.eye(P, dtype=bf)
    return [
        {
            "xTroll": np.ascontiguousarray(
                np.roll(xT, -BL * c, axis=1)[:, 0:CE]
            ),
            "thp": thp,
            "small": small,
            "sel": sel,
            "eye": eye,
        }
        for c in range(NCORES)
    ]


_CACHE = {}


def get_nc():
    if "nc" not in _CACHE:
        _CACHE["nc"] = build_nc()
    return _CACHE["nc"]


def kernel(x, theta, log_weight_scale, bias):
    from concourse.bass_utils import run_bass_kernel_spmd

    x = np.asarray(x, dtype=np.float32)
    theta = np.asarray(theta, dtype=np.float32)
    log_weight_scale = np.asarray(log_weight_scale, dtype=np.float32)
    bias = np.asarray(bias, dtype=np.float32)

    nc = get_nc()
    in_maps = make_in_maps(x, theta, log_weight_scale, bias)
    res = run_bass_kernel_spmd(nc, in_maps, list(range(NCORES))).results

    f = np.zeros((B, NK), dtype=np.float64)
    for c in range(NCORES):
        O = res[c]["fT"].astype(np.float64)  # [128, OUTW]
        fMA = O[:, 0:64]
        fA = O[:, 64:128]
        fB = O[:, 128:160]
        csM = O[:, 160:1696].reshape(NK, 8, 192)
        csA = O[:, 1696:1952].reshape(NK, 8, 32)
        csB = O[:, 1952:2080].reshape(NK, 4, 32)
        rows = (BL * c + np.arange(BL)) % B
        contrib = fMA.copy()
        if c >= 4:
            contrib[:, 0:32] -= fA[:, 0:32]
            contrib[:, 32:64] -= fB
        f[rows] += contrib.T
        f[(BL * c + 64 + np.arange(192)) % B] += csM.sum(axis=1).T
        vA = (BL * c + C0 + np.arange(32)) % B
        f[vA] += csA[:, 4:8].sum(axis=1).T
        if c < 4:
            f[vA] += csA[:, 0:4].sum(axis=1).T
            f[(BL * c + CM + np.arange(32)) % B] += csB.sum(axis=1).T
    f = f + bias[None, :] - 1.0
    return np.concatenate([x, f.astype(np.float32)], axis=1)
